# revision 1
# baseline (speedup 1.0000x reference)
"""Trainium2 Bass kernel for nn_Attention_40492951666725.

Full attention layer: qkv proj -> RoPE (interleaved pairs, rot dim 32) ->
softmax(QK^T)V -> out proj.  B=4, N=2048, DIM=1024, H=16, DH=64.

Sharding: 8 cores, core c handles batch b=c//2 and query-half c%2 (1024
query tokens, all 16 heads, full 2048-token K/V).  K/V projection is
computed redundantly by the two cores sharing a batch; no collectives.
The host rotates the token axis per core so the core's own query tokens
are always columns [0:1024] of xT (attention is permutation-invariant
over keys, so k/v/cos/sin just follow the same order).

Layouts (per core):
  xT   [DIM, 2048]  (host-transposed)   -> lhsT/rhs for projections
  q^T  [feat, 1024], k^T [feat, 2048]   feat on partitions
  S^T  [kj, qi]  (kj on partitions)     -> softmax via exp (no max-sub;
        scores are O(+-10) so fp32 exp is safe), denominator from a
        ones-column appended to V (M=65 AV matmuls), division applied to
        the [64, qi] head output (commutes with the PV sum).
  attn^T [inner, tok] -> out proj produces out [tok, DIM] directly.

RoPE: rotate_every_two(q) is a fixed feat-space linear map -> done with a
single [128,128] block-diagonal matmul (Rm), then q_rot = q*cos + (Rq)*sin
elementwise on DVE; pass-dims use cos=1/sin=0 so all 64 dims are uniform.
"""

import os
import numpy as np
import ml_dtypes

import concourse.bass as bass
from concourse import bacc
import concourse.tile as tile
from concourse import mybir, library_config
from concourse.bass_utils import run_bass_kernel_spmd

BF = ml_dtypes.bfloat16
bf16 = mybir.dt.bfloat16
f32 = mybir.dt.float32

B, N, DIM, H, DH, ROT = 4, 2048, 1024, 16, 64, 32
INNER = H * DH
NQ = N // 2            # query tokens per core
NCORES = 8
P = 128
KD = DIM // P          # 8 contraction tiles over model dim
NKT = N // P           # 16 kj partition tiles
HPB = H // 2           # 8 head-pair blocks

Exp = mybir.ActivationFunctionType.Exp

_CACHE = {}


def _build_rope_consts(sin, cos):
    """cos_pad/sin_pad [128, N] for one head-pair feat block, Rm [128,128].

    Uses the provided sin/cos tables [N, ROT]; pass-dims get cos=1/sin=0 so
    RoPE applies uniformly over all 64 head dims."""
    cos_pad = np.ones((P, N), np.float32)
    sin_pad = np.zeros((P, N), np.float32)
    for half in range(2):                                # two heads per block
        r0 = half * DH
        cos_pad[r0:r0 + ROT, :] = cos.T
        sin_pad[r0:r0 + ROT, :] = sin.T

    # Rm[dp, d]: out[d] = sum_dp Rm[dp, d] * q[dp]  == rotate_every_two(q)[d]
    Rm = np.zeros((P, P), np.float32)
    for half in range(2):
        r0 = half * DH
        for i in range(0, ROT, 2):
            Rm[r0 + i + 1, r0 + i] = -1.0                # out[2i]   = -q[2i+1]
            Rm[r0 + i, r0 + i + 1] = 1.0                 # out[2i+1] =  q[2i]
    return cos_pad, sin_pad, Rm


def _build_program():
    nc = bacc.Bacc(trn_type="TRN2")

    xkv_d = nc.dram_tensor("xkv", [DIM, N], bf16, kind="ExternalInput")
    wq_d = nc.dram_tensor("wq", [DIM, INNER], bf16, kind="ExternalInput")
    wk_d = nc.dram_tensor("wk", [DIM, INNER], bf16, kind="ExternalInput")
    wv_d = nc.dram_tensor("wv", [DIM, INNER], bf16, kind="ExternalInput")
    wo_d = nc.dram_tensor("wo", [INNER, DIM], bf16, kind="ExternalInput")
    cosk_d = nc.dram_tensor("cosk", [P, N], bf16, kind="ExternalInput")
    sink_d = nc.dram_tensor("sink", [P, N], bf16, kind="ExternalInput")
    rm_d = nc.dram_tensor("rm", [P, P], bf16, kind="ExternalInput")
    out_d = nc.dram_tensor("out", [NQ, DIM], f32, kind="ExternalOutput")

    with tile.TileContext(nc) as tc:
        with (
            tc.tile_pool(name="res", bufs=1) as res,          # kernel-lifetime tiles
            tc.tile_pool(name="kstream", bufs=2) as kstream,  # per-hp q/k tiles
            tc.tile_pool(name="wstream", bufs=1) as wstream,
            tc.tile_pool(name="pt", bufs=4) as ptp,           # P^T tiles
            tc.tile_pool(name="tmp", bufs=4) as tmp,          # rope DVE temps
            tc.tile_pool(name="small", bufs=2) as small,
            tc.tile_pool(name="ostage", bufs=3) as ostage,
            tc.tile_pool(name="psA", bufs=2, space="PSUM") as psA,    # [128,512] proj/outproj/swap
            tc.tile_pool(name="psS", bufs=2, space="PSUM") as psS,    # [128,1024] scores
            tc.tile_pool(name="psV", bufs=2, space="PSUM") as psV,    # [65,512] AV
        ):
            nc.gpsimd.load_library(library_config.attn)

            # ---- resident loads (small rope consts first, K-weights last) ----
            cosk = res.tile([P, N], bf16, tag="cosk")
            sink = res.tile([P, N], bf16, tag="sink")
            rm = res.tile([P, P], bf16, tag="rm")
            for t, d in ((rm, rm_d), (cosk, cosk_d), (sink, sink_d)):
                nc.sync.dma_start(t[:], d[:])
            xkv, wq, wk = [], [], []
            for k in range(KD):
                t = res.tile([P, N], bf16, tag=f"xkv{k}", name=f"xkv{k}")
                nc.sync.dma_start(t[:], xkv_d[k * P:(k + 1) * P, :])
                xkv.append(t)
                t = res.tile([P, DIM], bf16, tag=f"wq{k}", name=f"wq{k}")
                nc.sync.dma_start(t[:], wq_d[k * P:(k + 1) * P, :])
                wq.append(t)
            for k in range(KD):
                t = res.tile([P, DIM], bf16, tag=f"wk{k}", name=f"wk{k}")
                nc.sync.dma_start(t[:], wk_d[k * P:(k + 1) * P, :])
                wk.append(t)

            attnT = []
            for k in range(KD):
                attnT.append(res.tile([P, NQ], bf16, tag=f"attnT{k}", name=f"attnT{k}"))
            vaug = []
            for mt in range(NKT):
                vt = res.tile([P, HPB, 2, 65], bf16, tag=f"vaug{mt}", name=f"vaug{mt}")
                nc.vector.memset(vt[:, :, :, 64], 1.0)
                vaug.append(vt)

            def emit_vproj(bn):
                """Project V feats [bn*512, bn*512+512) = hp blocks 4bn..4bn+3."""
                wvt = []
                for k in range(KD):
                    t = wstream.tile([P, 512], bf16, tag=f"wv{k}", name=f"wv{k}")
                    nc.sync.dma_start(t[:], wv_d[k * P:(k + 1) * P,
                                                 bn * 512:(bn + 1) * 512])
                    wvt.append(t)
                for mt in range(NKT):
                    ps = psA.tile([P, 512], f32, tag="ps")
                    for k in range(KD):
                        nc.tensor.matmul(ps[:], xkv[k][:, mt * P:(mt + 1) * P],
                                         wvt[k][:],
                                         start=(k == 0), stop=(k == KD - 1))
                    nc.vector.tensor_copy(
                        vaug[mt][:, bn * 4:(bn + 1) * 4, :, 0:64],
                        ps[:].rearrange("p (b h d) -> p b h d", b=4, h=2))

            state = {}

            def emit_proj(hp):
                """Project+rope feat block hp (heads 2hp, 2hp+1)."""
                c0 = hp * P
                # --- q^T block: [128 feats, NQ]  (q tokens = xkv cols 0:NQ) ---
                qraw = kstream.tile([P, NQ], bf16, tag="qraw")
                for n in range(NQ // 512):
                    ps = psA.tile([P, 512], f32, tag="ps")
                    for k in range(KD):
                        nc.tensor.matmul(ps[:], wq[k][:, c0:c0 + P],
                                         xkv[k][:, n * 512:(n + 1) * 512],
                                         start=(k == 0), stop=(k == KD - 1))
                    nc.vector.tensor_copy(qraw[:, n * 512:(n + 1) * 512], ps[:])
                qrot = kstream.tile([P, NQ], bf16, tag="qrot")
                for n in range(NQ // 512):
                    sl = slice(n * 512, (n + 1) * 512)
                    psw = psA.tile([P, 512], f32, tag="ps")
                    nc.tensor.matmul(psw[:], rm[:], qraw[:, sl], start=True, stop=True)
                    t1 = tmp.tile([P, 512], bf16, tag="t1")
                    nc.vector.tensor_mul(t1[:], qraw[:, sl], cosk[:, sl])
                    t2 = tmp.tile([P, 512], bf16, tag="t2")
                    nc.vector.tensor_mul(t2[:], psw[:], sink[:, sl])
                    nc.vector.tensor_add(qrot[:, sl], t1[:], t2[:])
                # --- k^T block: [128 feats, N] ---
                kraw = kstream.tile([P, N], bf16, tag="kraw")
                for n in range(N // 512):
                    ps = psA.tile([P, 512], f32, tag="ps")
                    for k in range(KD):
                        nc.tensor.matmul(ps[:], wk[k][:, c0:c0 + P],
                                         xkv[k][:, n * 512:(n + 1) * 512],
                                         start=(k == 0), stop=(k == KD - 1))
                    nc.vector.tensor_copy(kraw[:, n * 512:(n + 1) * 512], ps[:])
                krot = kstream.tile([P, N], bf16, tag="krot")
                for n in range(N // 512):
                    sl = slice(n * 512, (n + 1) * 512)
                    psw = psA.tile([P, 512], f32, tag="ps")
                    nc.tensor.matmul(psw[:], rm[:], kraw[:, sl], start=True, stop=True)
                    t1 = tmp.tile([P, 512], bf16, tag="t1")
                    nc.vector.tensor_mul(t1[:], kraw[:, sl], cosk[:, sl])
                    t2 = tmp.tile([P, 512], bf16, tag="t2")
                    nc.vector.tensor_mul(t2[:], psw[:], sink[:, sl])
                    nc.vector.tensor_add(krot[:, sl], t1[:], t2[:])
                state[hp] = (qrot, krot)

            def emit_attn_qk(hp, half):
                qrot, krot = state[hp]
                hoff = half * DH
                pts = []
                for kt in range(NKT):
                    ps = psS.tile([P, NQ], f32, tag="s")
                    for qn in range(NQ // 512):
                        nc.tensor.matmul(
                            ps[:, qn * 512:(qn + 1) * 512],
                            krot[hoff:hoff + DH, kt * P:(kt + 1) * P],
                            qrot[hoff:hoff + DH, qn * 512:(qn + 1) * 512],
                            start=True, stop=True)
                    pt = ptp.tile([P, NQ], bf16, tag="pt")
                    nc.scalar.activation(pt[:], ps[:], Exp)
                    pts.append(pt)
                return pts

            def emit_attn_av(hp, half, pts):
                hoff = half * DH
                pvs = [psV.tile([65, 512], f32, tag="av", name="av")
                       for _ in range(NQ // 512)]
                for kt in range(NKT):
                    for qn in range(NQ // 512):
                        nc.tensor.matmul(pvs[qn][:], vaug[kt][:, hp, half, :],
                                         pts[kt][:, qn * 512:(qn + 1) * 512],
                                         start=(kt == 0), stop=(kt == NKT - 1))
                for qn in range(NQ // 512):
                    sl = slice(qn * 512, (qn + 1) * 512)
                    pv = pvs[qn]
                    rec = small.tile([1, 512], f32, tag="rec")
                    nc.vector.reciprocal(rec[:], pv[64:65, :])
                    recb = small.tile([64, 512], f32, tag="recb")
                    nc.gpsimd.partition_broadcast(recb[:], rec[:])
                    nc.vector.tensor_mul(attnT[hp][hoff:hoff + DH, sl],
                                         pv[0:64, :], recb[:])

            def prefetch_wo(n):
                wot = []
                for k in range(KD):
                    t = wstream.tile([P, 512], bf16, tag=f"wo{n}_{k}",
                                     name=f"wo{n}_{k}", bufs=1)
                    nc.sync.dma_start(t[:], wo_d[k * P:(k + 1) * P,
                                                 n * 512:(n + 1) * 512])
                    wot.append(t)
                return wot

            wo_pre = {}
            emit_proj(0)
            emit_vproj(0)
            for hp in range(HPB):
                pts0 = emit_attn_qk(hp, 0)
                emit_attn_av(hp, 0, pts0)
                if hp + 1 < HPB:
                    emit_proj(hp + 1)
                pts1 = emit_attn_qk(hp, 1)
                if hp == 2:
                    emit_vproj(1)
                if hp == HPB - 2:
                    wo_pre[0] = prefetch_wo(0)
                if hp == HPB - 1:
                    wo_pre[1] = prefetch_wo(1)
                emit_attn_av(hp, 1, pts1)
                state.pop(hp)

            # ---- out projection: out[tok, DIM] = attnT.T @ Wout ----
            for n in range(DIM // 512):
                wot = wo_pre[n]
                for mt in range(NQ // P):
                    ps = psA.tile([P, 512], f32, tag="ps")
                    for k in range(KD):
                        nc.tensor.matmul(ps[:], attnT[k][:, mt * P:(mt + 1) * P],
                                         wot[k][:],
                                         start=(k == 0), stop=(k == KD - 1))
                    st = ostage.tile([P, 512], f32, tag="ost")
                    nc.scalar.copy(st[:], ps[:])
                    nc.sync.dma_start(
                        out_d[mt * P:(mt + 1) * P, n * 512:(n + 1) * 512], st[:])

    nc.compile()
    return nc


def _prep_inputs(x, sin, cos, Wqkv, Wout):
    """Host-side sharding/layout prep. Returns in_maps list for 8 cores."""
    x = np.asarray(x, np.float32)
    Wqkv = np.asarray(Wqkv, np.float32)
    Wout = np.asarray(Wout, np.float32)
    scale = DH ** -0.5
    wq = (Wqkv[:, :INNER] * scale).astype(BF)
    wk = Wqkv[:, INNER:2 * INNER].astype(BF)
    wv = Wqkv[:, 2 * INNER:].astype(BF)
    wo = Wout.astype(BF)
    cos_pad, sin_pad, Rm = _build_rope_consts(
        np.asarray(sin, np.float32), np.asarray(cos, np.float32))
    rm = Rm.astype(BF)

    in_maps = []
    for c in range(NCORES):
        b, half = divmod(c, 2)
        xT = np.ascontiguousarray(x[b].T)                          # [DIM, N]
        ck, sk = cos_pad, sin_pad
        if half == 1:        # rotate tokens so this core's queries come first
            xT = np.concatenate([xT[:, NQ:], xT[:, :NQ]], axis=1)
            ck = np.concatenate([ck[:, NQ:], ck[:, :NQ]], axis=1)
            sk = np.concatenate([sk[:, NQ:], sk[:, :NQ]], axis=1)
        in_maps.append({
            "xkv": np.ascontiguousarray(xT).astype(BF),
            "wq": wq, "wk": wk, "wv": wv, "wo": wo,
            "cosk": np.ascontiguousarray(ck).astype(BF),
            "sink": np.ascontiguousarray(sk).astype(BF),
            "rm": rm,
        })
    return in_maps


LAST_RESULTS = None


def kernel(x, sin, cos, Wqkv, Wout):
    global LAST_RESULTS
    if "nc" not in _CACHE:
        _CACHE["nc"] = _build_program()
    nc = _CACHE["nc"]
    in_maps = _prep_inputs(x, sin, cos, Wqkv, Wout)
    trace = bool(int(os.environ.get("KERNEL_TRACE", "0")))
    try:
        res = run_bass_kernel_spmd(nc, in_maps, core_ids=list(range(NCORES)),
                                   trace=trace)
    except (ImportError, ModuleNotFoundError):
        # NTFF profiling hook unavailable in this environment
        res = run_bass_kernel_spmd(nc, in_maps, core_ids=list(range(NCORES)),
                                   trace=False)
    LAST_RESULTS = res
    out = np.empty((B, N, DIM), np.float32)
    for c in range(NCORES):
        b, half = divmod(c, 2)
        out[b, half * NQ:(half + 1) * NQ, :] = res.results[c]["out"]
    return out



# revision 25
# speedup vs baseline: 1.1160x; 1.1160x over previous
"""Trainium2 Bass kernel for nn_Attention_40492951666725.

Full attention layer: qkv proj -> RoPE (interleaved pairs, rot dim 32) ->
softmax(QK^T)V -> out proj.  B=4, N=2048, DIM=1024, H=16, DH=64.

Sharding: 8 cores, core c handles batch b=c//2 and query-half c%2 (1024
query tokens, all 16 heads, full 2048-token K/V).  K/V projection is
computed redundantly by the two cores sharing a batch; no collectives.
The host rotates the token axis per core so the core's own query tokens
are always columns [0:1024] of xT (attention is permutation-invariant
over keys, so k/v/cos/sin just follow the same order).

Layouts (per core):
  xT   [DIM, 2048]  (host-transposed)   -> lhsT/rhs for projections
  q^T  [feat, 1024], k^T [feat, 2048]   feat on partitions
  S^T  [kj, qi]  (kj on partitions)     -> softmax via exp (no max-sub;
        scores are O(+-10) so fp32 exp is safe), denominator from a
        ones-column appended to V.
  AV   uses P^T slices as the stationary operand (qi on feats) so the PE
        runs at full 128x128 utilization: out [qi, 65] per qi-tile, the
        denominator lands on the same partition as its row (per-partition
        tensor_scalar divide), then a PE transpose returns the head block
        to attn^T [inner, tok] for the out projection.
  attn^T [inner, tok] -> out proj produces out [tok, DIM] directly.

Scheduling: the Act engine's exp stream (~1.01us per [128,1024] tile) is
the metronome of the attention phase.  The emitter interleaves, per kt
"beat": the QK matmuls of pair p, one AV chain of pair p-1, and filler
chunklets (projection / V-projection work pulled from generators), so
the PE always has independent work while Act chews through the exps.

RoPE: rotate_every_two(q) is a fixed feat-space linear map -> done with a
single [128,128] block-diagonal matmul (Rm), then q_rot = q*cos + (Rq)*sin
elementwise on DVE; pass-dims use cos=1/sin=0 so all 64 dims are uniform.
"""

import os
import numpy as np
import ml_dtypes

import concourse.bass as bass
from concourse import bacc
import concourse.tile as tile
from concourse import mybir, library_config
from concourse.bass_utils import run_bass_kernel_spmd

BF = ml_dtypes.bfloat16
bf16 = mybir.dt.bfloat16
f32 = mybir.dt.float32

B, N, DIM, H, DH, ROT = 4, 2048, 1024, 16, 64, 32
INNER = H * DH         # 1024
NQ = N // 2            # query tokens per core
NCORES = 8
P = 128
KD = DIM // P          # 8 contraction tiles over model dim
NKT = N // P           # 16 kj partition tiles
HPB = H // 2           # 8 head-pair blocks
NQT = NQ // P          # 8 qi tiles

Exp = mybir.ActivationFunctionType.Exp

_CACHE = {}


def _build_rope_consts(sin, cos):
    """cos_pad/sin_pad [128, N] for one head-pair feat block, Rm [128,128].

    Uses the provided sin/cos tables [N, ROT]; pass-dims get cos=1/sin=0 so
    RoPE applies uniformly over all 64 head dims."""
    cos_pad = np.ones((P, N), np.float32)
    sin_pad = np.zeros((P, N), np.float32)
    for half in range(2):                                # two heads per block
        r0 = half * DH
        cos_pad[r0:r0 + ROT, :] = cos.T
        sin_pad[r0:r0 + ROT, :] = sin.T

    # Rm[dp, d]: out[d] = sum_dp Rm[dp, d] * q[dp]  == rotate_every_two(q)[d]
    Rm = np.zeros((P, P), np.float32)
    for half in range(2):
        r0 = half * DH
        for i in range(0, ROT, 2):
            Rm[r0 + i + 1, r0 + i] = -1.0                # out[2i]   = -q[2i+1]
            Rm[r0 + i, r0 + i + 1] = 1.0                 # out[2i+1] =  q[2i]
    return cos_pad, sin_pad, Rm


def _build_program():
    nc = bacc.Bacc(trn_type="TRN2")

    xkv_d = nc.dram_tensor("xkv", [DIM, N], bf16, kind="ExternalInput")
    wq_d = nc.dram_tensor("wq", [DIM, INNER], bf16, kind="ExternalInput")
    wk_d = nc.dram_tensor("wk", [DIM, INNER], bf16, kind="ExternalInput")
    wv_d = nc.dram_tensor("wv", [DIM, INNER], bf16, kind="ExternalInput")
    wo_d = nc.dram_tensor("wo", [INNER, DIM], bf16, kind="ExternalInput")
    cosk_d = nc.dram_tensor("cosk", [P, N], bf16, kind="ExternalInput")
    sink_d = nc.dram_tensor("sink", [P, N], bf16, kind="ExternalInput")
    rm_d = nc.dram_tensor("rm", [P, P], bf16, kind="ExternalInput")
    ident_d = nc.dram_tensor("ident", [P, P], bf16, kind="ExternalInput")
    out_d = nc.dram_tensor("out", [NQ, DIM], f32, kind="ExternalOutput")

    # [p, k, c] view of a [DIM, INNER]-ish weight matrix: row k*128+p, col c
    def blocked(d, cols):
        return d.rearrange("(k p) c -> p k c", k=KD, p=P)[:, :, cols]

    with tile.TileContext(nc) as tc:
        with (
            tc.tile_pool(name="res", bufs=1) as res,          # kernel-lifetime tiles
            tc.tile_pool(name="wqk", bufs=2) as wqk,          # per-hp wq/wk blocks
            tc.tile_pool(name="rawp", bufs=1) as rawp,        # pre-rope q/k
            tc.tile_pool(name="rotp", bufs=3) as rotp,        # post-rope q/k
            tc.tile_pool(name="wstream", bufs=1) as wstream,  # wv / wo column streams
            tc.tile_pool(name="ptp", bufs=32) as ptp,         # P^T tiles (2 pairs live)
            tc.tile_pool(name="tmp", bufs=2) as tmp,          # rope/div DVE temps
            tc.tile_pool(name="small", bufs=4) as small,
            tc.tile_pool(name="ostage", bufs=2) as ostage,
            tc.tile_pool(name="psA", bufs=2, space="PSUM") as psA,  # [128,512] proj/outproj/transp
            tc.tile_pool(name="psS", bufs=2, space="PSUM") as psS,  # [128,1024] scores
            tc.tile_pool(name="psV", bufs=1, space="PSUM") as psV,  # 2x[128,65] AV accum
        ):
            # ---- resident loads (what the first PE work needs comes first) --
            wload = {}

            def load_wqk(hp):
                c0 = hp * P
                wqb = wqk.tile([P, KD, P], bf16, tag="wqb", name=f"wqb{hp}")
                nc.sync.dma_start(wqb[:], blocked(wq_d, slice(c0, c0 + P)))
                wkb = wqk.tile([P, KD, P], bf16, tag="wkb", name=f"wkb{hp}")
                nc.sync.dma_start(wkb[:], blocked(wk_d, slice(c0, c0 + P)))
                wload[hp] = (wqb, wkb)

            load_wqk(0)
            # The startup is DMA-paced: triggers are serialized per engine
            # queue (~0.6us each) and the DMA device drains in trigger order,
            # so loads are issued in consumption order of the column-gated
            # startup emission below, split across the SP and Act queues.
            xkv = [res.tile([P, N], bf16, tag=f"xkv{k}", name=f"xkv{k}")
                   for k in range(KD)]
            wv0 = []
            for k in range(KD):      # Act queue: wv (vproj) first, then n1/n3
                t = wstream.tile([P, 512], bf16, tag=f"wv{k}", name=f"wv{k}")
                nc.scalar.dma_start(t[:], wv_d[k * P:(k + 1) * P, 0:512])
                wv0.append(t)
            for k in range(KD):      # SP queue: xkv col chunk 0
                nc.sync.dma_start(xkv[k][:, 0:512],
                                  xkv_d[k * P:(k + 1) * P, 0:512])
            rm = res.tile([P, P], bf16, tag="rm")
            ident = res.tile([P, P], bf16, tag="ident")
            nc.sync.dma_start(rm[:], rm_d[:])
            nc.sync.dma_start(ident[:], ident_d[:])
            for k in range(KD):      # Act: col chunk 1
                nc.scalar.dma_start(xkv[k][:, 512:1024],
                                    xkv_d[k * P:(k + 1) * P, 512:1024])
            cosk = res.tile([P, N], bf16, tag="cosk")
            sink = res.tile([P, N], bf16, tag="sink")
            nc.sync.dma_start(cosk[:], cosk_d[:])
            nc.sync.dma_start(sink[:], sink_d[:])
            for k in range(KD):      # SP: col chunk 2
                nc.sync.dma_start(xkv[k][:, 1024:1536],
                                  xkv_d[k * P:(k + 1) * P, 1024:1536])
            for k in range(KD):      # Act: col chunk 3
                nc.scalar.dma_start(xkv[k][:, 1536:N],
                                    xkv_d[k * P:(k + 1) * P, 1536:N])
            load_wqk(1)

            attnT = []
            for k in range(KD):
                attnT.append(res.tile([P, NQ], bf16, tag=f"attnT{k}", name=f"attnT{k}"))
            vaug = []
            for mt in range(NKT):
                vt = res.tile([P, HPB, 2, 65], bf16, tag=f"vaug{mt}", name=f"vaug{mt}")
                nc.vector.memset(vt[:, :, :, 64], 1.0)
                vaug.append(vt)

            state = {}

            # ---- projection chunk emitters --------------------------------
            def emit_proj_chunk(wb, dst, n):
                """One 512-token projection chain into dst (qraw/kraw)."""
                ps = psA.tile([P, 512], f32, tag="ps")
                for k in range(KD):
                    nc.tensor.matmul(ps[:], wb[:, k, :],
                                     xkv[k][:, n * 512:(n + 1) * 512],
                                     start=(k == 0), stop=(k == KD - 1))
                nc.vector.tensor_copy(dst[:, n * 512:(n + 1) * 512], ps[:])

            def emit_rope_chunk(raw, rot, n):
                sl = slice(n * 512, (n + 1) * 512)
                psw = psA.tile([P, 512], f32, tag="ps")
                nc.tensor.matmul(psw[:], rm[:], raw[:, sl], start=True, stop=True)
                nc.vector.tensor_mul(rot[:, sl], raw[:, sl], cosk[:, sl])
                t2 = tmp.tile([P, 512], bf16, tag="t2")
                nc.vector.tensor_mul(t2[:], psw[:], sink[:, sl])
                nc.vector.tensor_add(rot[:, sl], rot[:, sl], t2[:])

            def emit_vproj_mt(wvt, bn, mt):
                """V-projection for kj tile mt, feat block bn."""
                ps = psA.tile([P, 512], f32, tag="ps")
                for k in range(KD):
                    nc.tensor.matmul(ps[:], xkv[k][:, mt * P:(mt + 1) * P],
                                     wvt[k][:],
                                     start=(k == 0), stop=(k == KD - 1))
                nc.vector.tensor_copy(
                    vaug[mt][:, bn * 4:(bn + 1) * 4, :, 0:64],
                    ps[:].rearrange("p (b h d) -> p b h d", b=4, h=2))

            # ---- filler generators: yield ~PE cycles emitted so far --------
            def gen_proj(hp):
                """Project+rope feat block hp (heads 2hp, 2hp+1)."""
                if hp + 1 < HPB and hp + 1 not in wload:
                    load_wqk(hp + 1)               # prefetch next block's weights
                wqb, wkb = wload.pop(hp)
                qraw = rawp.tile([P, NQ], bf16, tag="qraw")
                for n in range(NQ // 512):
                    emit_proj_chunk(wqb, qraw, n)
                    yield 4096
                qrot = rotp.tile([P, NQ], bf16, tag="qrot")
                for n in range(NQ // 512):
                    emit_rope_chunk(qraw, qrot, n)
                    yield 512
                kraw = rawp.tile([P, N], bf16, tag="kraw")
                for n in range(N // 512):
                    emit_proj_chunk(wkb, kraw, n)
                    yield 4096
                krot = rotp.tile([P, N], bf16, tag="krot")
                for n in range(N // 512):
                    emit_rope_chunk(kraw, krot, n)
                    yield 512
                state[hp] = (qrot, krot)

            def gen_vproj(bn):
                """Project V feats [bn*512, bn*512+512) = hp blocks 4bn..4bn+3."""
                wvt = []
                for k in range(KD):
                    t = wstream.tile([P, 512], bf16, tag=f"wv{k}", name=f"wv{k}")
                    nc.sync.dma_start(t[:], wv_d[k * P:(k + 1) * P,
                                                 bn * 512:(bn + 1) * 512])
                    wvt.append(t)
                for mt in range(NKT):
                    emit_vproj_mt(wvt, bn, mt)
                    yield 4096

            def gen_wo_prefetch(n, tags):
                wot = []
                for k in range(KD):
                    t = wstream.tile([P, 512], bf16, tag=tags.format(k=k),
                                     name=f"wo{n}_{k}")
                    nc.sync.dma_start(t[:], wo_d[k * P:(k + 1) * P,
                                                 n * 512:(n + 1) * 512])
                    wot.append(t)
                wo_pre[n] = wot
                yield 0

            wo_pre = {}

            # ---- filler scheduler -----------------------------------------
            # (generator, deadline_slot): generator must be fully drained
            # before slot `deadline_slot` begins emitting.
            fillers = [
                (gen_proj(1), 2), (gen_proj(2), 4), (gen_proj(3), 6),
                (gen_proj(4), 8), (gen_vproj(1), 9), (gen_proj(5), 10),
                (gen_proj(6), 12), (gen_proj(7), 14),
                (gen_wo_prefetch(0, "wo{k}"), 14),
                (gen_wo_prefetch(1, "wv{k}"), 15),   # reuse dead wv slots
            ]
            # total filler PE cycles: 7 proj x 27648 + vproj 65536
            filler_left = [7 * 27648 + 65536]

            credit = [0.0]

            def drive_filler(budget):
                # credit paces coarse chunks: a 4096-cycle chunk emitted on a
                # 1012-cycle budget leaves negative credit, skipping beats
                # until repaid.
                credit[0] += budget
                while fillers and credit[0] > 0:
                    gen, _ = fillers[0]
                    try:
                        c = next(gen)
                        credit[0] -= c
                        filler_left[0] -= c
                    except StopIteration:
                        fillers.pop(0)

            def drain_due(slot):
                while fillers and fillers[0][1] <= slot:
                    gen, _ = fillers[0]
                    for c in gen:
                        filler_left[0] -= c
                    fillers.pop(0)

            # ---- attention building blocks --------------------------------
            def emit_qk_beat(hp, half, kt):
                qrot, krot = state[hp]
                hoff = half * DH
                ps = psS.tile([P, NQ], f32, tag="s")
                for qn in range(NQ // 512):
                    nc.tensor.matmul(
                        ps[:, qn * 512:(qn + 1) * 512],
                        krot[hoff:hoff + DH, kt * P:(kt + 1) * P],
                        qrot[hoff:hoff + DH, qn * 512:(qn + 1) * 512],
                        start=True, stop=True)
                pt = ptp.tile([P, NQ], bf16, tag="pt")
                nc.scalar.activation(pt[:], ps[:], Exp)
                return pt

            def emit_av_chain(hp, half, pts, qt):
                """One qi-tile AV chain + divide + transpose into attn^T."""
                hoff = half * DH
                pv = psV.tile([P, 65], f32, tag=f"av{qt % 2}", name="av")
                for kt in range(NKT):
                    nc.tensor.matmul(pv[:], pts[kt][:, qt * P:(qt + 1) * P],
                                     vaug[kt][:, hp, half, :],
                                     start=(kt == 0), stop=(kt == NKT - 1))
                rec = small.tile([P, 1], f32, tag="rec")
                nc.vector.reciprocal(rec[:], pv[:, 64:65])
                dsb = tmp.tile([P, DH], bf16, tag="dsb")
                nc.vector.tensor_scalar_mul(dsb[:], pv[:, 0:DH], rec[:])
                # transpose via plain matmul against the identity: out[f, c]
                # = sum_p dsb[p, f] * I[p, c] = dsb[c, f].  Same PE cost as
                # transpose mode, but no value constraint on the rhs.
                pst = psA.tile([P, 512], f32, tag="ps")
                pstv = pst[0:DH, 0:P]
                nc.tensor.matmul(pstv, dsb[:], ident[:], start=True, stop=True)
                nc.vector.tensor_copy(
                    attnT[hp][hoff:hoff + DH, qt * P:(qt + 1) * P], pstv)

            # ---- emission: startup, pipelined slots, tail -----------------
            # Column-gated startup: proj(0) and vproj(0) interleaved in the
            # order their xkv column chunks arrive from DRAM, so the PE
            # chases the DMA stream instead of waiting for all of xkv.
            load_wqk(2)
            wqb0, wkb0 = wload.pop(0)
            qraw = rawp.tile([P, NQ], bf16, tag="qraw")
            kraw = rawp.tile([P, N], bf16, tag="kraw")
            for n in range(N // 512):
                if n < NQ // 512:
                    emit_proj_chunk(wqb0, qraw, n)
                emit_proj_chunk(wkb0, kraw, n)
                for mt in range(4 * n, 4 * n + 4):
                    emit_vproj_mt(wv0, 0, mt)
                if n == 1:
                    qrot = rotp.tile([P, NQ], bf16, tag="qrot")
                    for nn in range(NQ // 512):
                        emit_rope_chunk(qraw, qrot, nn)
            krot = rotp.tile([P, N], bf16, tag="krot")
            for n in range(N // 512):
                emit_rope_chunk(kraw, krot, n)
            state[0] = (qrot, krot)

            prev = None                # (hp, half, pts) of pair p-1
            for p in range(2 * HPB):
                hp, half = divmod(p, 2)
                drain_due(p)
                slots_left = 2 * HPB - p
                budget = max(0, filler_left[0] // (slots_left * NKT) + 1)
                # prepump: QK kt0/kt1 reuse psS buffers still draining through
                # the previous pair's last exps; filler work is independent
                # of that, so spend some here to absorb the wait
                drive_filler(3000)
                pts = []
                for kt in range(NKT):
                    pts.append(emit_qk_beat(hp, half, kt))
                    if prev is not None and kt % 2 == 1:
                        emit_av_chain(prev[0], prev[1], prev[2], kt // 2)
                    drive_filler(budget)
                if half == 1:
                    state.pop(hp)      # krot/qrot no longer needed after QK
                prev = (hp, half, pts)
            drain_due(99)              # whatever filler is left (wo prefetch)
            for qt in range(NQT):      # last pair's AV
                emit_av_chain(prev[0], prev[1], prev[2], qt)

            # ---- out projection: out[tok, DIM] = attnT.T @ Wout -----------
            for n in range(DIM // 512):
                wot = wo_pre[n]
                for mt in range(NQ // P):
                    ps = psA.tile([P, 512], f32, tag="ps")
                    for k in range(KD):
                        nc.tensor.matmul(ps[:], attnT[k][:, mt * P:(mt + 1) * P],
                                         wot[k][:],
                                         start=(k == 0), stop=(k == KD - 1))
                    st = ostage.tile([P, 512], f32, tag="ost")
                    nc.scalar.copy(st[:], ps[:])
                    nc.sync.dma_start(
                        out_d[mt * P:(mt + 1) * P, n * 512:(n + 1) * 512], st[:])

    nc.compile()
    return nc


def _prep_inputs(x, sin, cos, Wqkv, Wout):
    """Host-side sharding/layout prep. Returns in_maps list for 8 cores."""
    x = np.asarray(x, np.float32)
    Wqkv = np.asarray(Wqkv, np.float32)
    Wout = np.asarray(Wout, np.float32)
    scale = DH ** -0.5
    wq = (Wqkv[:, :INNER] * scale).astype(BF)
    wk = Wqkv[:, INNER:2 * INNER].astype(BF)
    wv = Wqkv[:, 2 * INNER:].astype(BF)
    wo = Wout.astype(BF)
    cos_pad, sin_pad, Rm = _build_rope_consts(
        np.asarray(sin, np.float32), np.asarray(cos, np.float32))
    rm = Rm.astype(BF)
    ident = np.eye(P, dtype=np.float32).astype(BF)

    in_maps = []
    for c in range(NCORES):
        b, half = divmod(c, 2)
        xT = np.ascontiguousarray(x[b].T)                          # [DIM, N]
        ck, sk = cos_pad, sin_pad
        if half == 1:        # rotate tokens so this core's queries come first
            xT = np.concatenate([xT[:, NQ:], xT[:, :NQ]], axis=1)
            ck = np.concatenate([ck[:, NQ:], ck[:, :NQ]], axis=1)
            sk = np.concatenate([sk[:, NQ:], sk[:, :NQ]], axis=1)
        in_maps.append({
            "xkv": np.ascontiguousarray(xT).astype(BF),
            "wq": wq, "wk": wk, "wv": wv, "wo": wo,
            "cosk": np.ascontiguousarray(ck).astype(BF),
            "sink": np.ascontiguousarray(sk).astype(BF),
            "rm": rm,
            "ident": ident,
        })
    return in_maps


LAST_RESULTS = None


def kernel(x, sin, cos, Wqkv, Wout):
    global LAST_RESULTS
    if "nc" not in _CACHE:
        _CACHE["nc"] = _build_program()
    nc = _CACHE["nc"]
    in_maps = _prep_inputs(x, sin, cos, Wqkv, Wout)
    trace = bool(int(os.environ.get("KERNEL_TRACE", "0")))
    try:
        res = run_bass_kernel_spmd(nc, in_maps, core_ids=list(range(NCORES)),
                                   trace=trace)
    except (ImportError, ModuleNotFoundError):
        # NTFF profiling hook unavailable in this environment
        res = run_bass_kernel_spmd(nc, in_maps, core_ids=list(range(NCORES)),
                                   trace=False)
    LAST_RESULTS = res
    out = np.empty((B, N, DIM), np.float32)
    for c in range(NCORES):
        b, half = divmod(c, 2)
        out[b, half * NQ:(half + 1) * NQ, :] = res.results[c]["out"]
    return out


# revision 31
# speedup vs baseline: 1.1685x; 1.0470x over previous
"""Trainium2 Bass kernel for nn_Attention_40492951666725.

Full attention layer: qkv proj -> RoPE (interleaved pairs, rot dim 32) ->
softmax(QK^T)V -> out proj.  B=4, N=2048, DIM=1024, H=16, DH=64.

Sharding: 8 cores, core c handles batch b=c//2 and query-half c%2 (1024
query tokens, all 16 heads, full 2048-token K/V).  K/V projection is
computed redundantly by the two cores sharing a batch; no collectives.
The host rotates the token axis per core so the core's own query tokens
are always columns [0:1024] of xT (attention is permutation-invariant
over keys, so k/v/cos/sin just follow the same order).

Layouts (per core):
  xT   [DIM, 2048]  (host-transposed)   -> lhsT/rhs for projections
  q^T  [feat, 1024], k^T [feat, 2048]   feat on partitions
  S^T  [kj, qi]  (kj on partitions)     -> softmax via exp (no max-sub;
        scores are O(+-10) so fp32 exp is safe), denominator from a
        ones-column appended to V.
  AV   uses P^T slices as the stationary operand (qi on feats) so the PE
        runs at full 128x128 utilization: out [qi, 65] per qi-tile, the
        denominator lands on the same partition as its row (per-partition
        tensor_scalar divide), then a PE transpose returns the head block
        to attn^T [inner, tok] for the out projection.
  attn^T [inner, tok] -> out proj produces out [tok, DIM] directly.

Scheduling: the Act engine's exp stream (~1.01us per [128,1024] tile) is
the metronome of the attention phase.  The emitter interleaves, per kt
"beat": the QK matmuls of pair p, one AV chain of pair p-1, and filler
chunklets (projection / V-projection work pulled from generators), so
the PE always has independent work while Act chews through the exps.

RoPE: rotate_every_two(q) is a fixed feat-space linear map -> done with a
single [128,128] block-diagonal matmul (Rm), then q_rot = q*cos + (Rq)*sin
elementwise on DVE; pass-dims use cos=1/sin=0 so all 64 dims are uniform.
"""

import os
import numpy as np
import ml_dtypes

import concourse.bass as bass
from concourse import bacc
import concourse.tile as tile
from concourse import mybir, library_config
from concourse.bass_utils import run_bass_kernel_spmd

BF = ml_dtypes.bfloat16
bf16 = mybir.dt.bfloat16
f32 = mybir.dt.float32

B, N, DIM, H, DH, ROT = 4, 2048, 1024, 16, 64, 32
INNER = H * DH         # 1024
NQ = N // 2            # query tokens per core
NCORES = 8
P = 128
KD = DIM // P          # 8 contraction tiles over model dim
NKT = N // P           # 16 kj partition tiles
HPB = H // 2           # 8 head-pair blocks
NQT = NQ // P          # 8 qi tiles

Exp = mybir.ActivationFunctionType.Exp

_CACHE = {}


def _build_rope_consts(sin, cos):
    """cos_pad/sin_pad [128, N] for one head-pair feat block, Rm [128,128].

    Uses the provided sin/cos tables [N, ROT]; pass-dims get cos=1/sin=0 so
    RoPE applies uniformly over all 64 head dims."""
    cos_pad = np.ones((P, N), np.float32)
    sin_pad = np.zeros((P, N), np.float32)
    for half in range(2):                                # two heads per block
        r0 = half * DH
        cos_pad[r0:r0 + ROT, :] = cos.T
        sin_pad[r0:r0 + ROT, :] = sin.T

    # Rm[dp, d]: out[d] = sum_dp Rm[dp, d] * q[dp]  == rotate_every_two(q)[d]
    Rm = np.zeros((P, P), np.float32)
    for half in range(2):
        r0 = half * DH
        for i in range(0, ROT, 2):
            Rm[r0 + i + 1, r0 + i] = -1.0                # out[2i]   = -q[2i+1]
            Rm[r0 + i, r0 + i + 1] = 1.0                 # out[2i+1] =  q[2i]
    return cos_pad, sin_pad, Rm


def _build_program():
    nc = bacc.Bacc(trn_type="TRN2")

    xkv_d = nc.dram_tensor("xkv", [DIM, N], bf16, kind="ExternalInput")
    wq_d = nc.dram_tensor("wq", [DIM, INNER], bf16, kind="ExternalInput")
    wk_d = nc.dram_tensor("wk", [DIM, INNER], bf16, kind="ExternalInput")
    wv_d = nc.dram_tensor("wv", [DIM, INNER], bf16, kind="ExternalInput")
    wo_d = nc.dram_tensor("wo", [INNER, DIM], bf16, kind="ExternalInput")
    cosk_d = nc.dram_tensor("cosk", [P, N], bf16, kind="ExternalInput")
    sink_d = nc.dram_tensor("sink", [P, N], bf16, kind="ExternalInput")
    rm_d = nc.dram_tensor("rm", [P, P], bf16, kind="ExternalInput")
    ident_d = nc.dram_tensor("ident", [P, P], bf16, kind="ExternalInput")
    out_d = nc.dram_tensor("out", [NQ, DIM], f32, kind="ExternalOutput")

    # [p, k, c] view of a [DIM, INNER]-ish weight matrix: row k*128+p, col c
    def blocked(d, cols):
        return d.rearrange("(k p) c -> p k c", k=KD, p=P)[:, :, cols]

    with tile.TileContext(nc) as tc:
        with (
            tc.tile_pool(name="res", bufs=1) as res,          # kernel-lifetime tiles
            tc.tile_pool(name="wqk", bufs=2) as wqk,          # per-hp wq/wk blocks
            tc.tile_pool(name="rawp", bufs=1) as rawp,        # pre-rope q/k
            tc.tile_pool(name="rotp", bufs=3) as rotp,        # post-rope q/k
            tc.tile_pool(name="wstream", bufs=1) as wstream,  # wv / wo column streams
            tc.tile_pool(name="ptp", bufs=28) as ptp,         # P^T tiles (~2 pairs live)
            tc.tile_pool(name="tmp", bufs=2) as tmp,          # rope/div DVE temps
            tc.tile_pool(name="dsbp", bufs=16) as dsbp,       # divided AV halves
            tc.tile_pool(name="small", bufs=4) as small,
            tc.tile_pool(name="ostage", bufs=2) as ostage,
            tc.tile_pool(name="psA", bufs=2, space="PSUM") as psA,  # [128,512] proj/outproj/transp
            tc.tile_pool(name="psS", bufs=2, space="PSUM") as psS,  # [128,1024] scores
            tc.tile_pool(name="psV", bufs=1, space="PSUM") as psV,  # 2x[128,65] AV accum
        ):
            # ---- resident loads (what the first PE work needs comes first) --
            wload = {}

            def load_wqk(hp):
                c0 = hp * P
                wqb = wqk.tile([P, KD, P], bf16, tag="wqb", name=f"wqb{hp}")
                nc.sync.dma_start(wqb[:], blocked(wq_d, slice(c0, c0 + P)))
                wkb = wqk.tile([P, KD, P], bf16, tag="wkb", name=f"wkb{hp}")
                nc.sync.dma_start(wkb[:], blocked(wk_d, slice(c0, c0 + P)))
                wload[hp] = (wqb, wkb)

            def load_wqk0():
                wqb = wqk.tile([P, KD, P], bf16, tag="wqb", name="wqb0")
                nc.sync.dma_start(wqb[:, 0, :], blocked(wq_d, slice(0, P))[:, 0, :])
                nc.sync.dma_start(wqb[:, 1:, :], blocked(wq_d, slice(0, P))[:, 1:, :])
                wkb = wqk.tile([P, KD, P], bf16, tag="wkb", name="wkb0")
                nc.sync.dma_start(wkb[:], blocked(wk_d, slice(0, P)))
                wload[0] = (wqb, wkb)

            load_wqk0()
            # The startup is DMA-paced: triggers are serialized per engine
            # queue (~0.6us each) and the DMA device drains in trigger order,
            # so loads are issued in consumption order of the column-gated
            # startup emission below, split across the SP and Act queues.
            xkv = [res.tile([P, N], bf16, tag=f"xkv{k}", name=f"xkv{k}")
                   for k in range(KD)]
            wv0 = []
            for k in range(KD):      # Act queue: wv (vproj) first, then n1/n3
                t = wstream.tile([P, 512], bf16, tag=f"wv{k}", name=f"wv{k}")
                nc.scalar.dma_start(t[:], wv_d[k * P:(k + 1) * P, 0:512])
                wv0.append(t)
            for k in range(KD):      # SP queue: xkv col chunk 0
                nc.sync.dma_start(xkv[k][:, 0:512],
                                  xkv_d[k * P:(k + 1) * P, 0:512])
            rm = res.tile([P, P], bf16, tag="rm")
            ident = res.tile([P, P], bf16, tag="ident")
            nc.sync.dma_start(rm[:], rm_d[:])
            nc.sync.dma_start(ident[:], ident_d[:])
            for k in range(KD):      # Act: col chunk 1
                nc.scalar.dma_start(xkv[k][:, 512:1024],
                                    xkv_d[k * P:(k + 1) * P, 512:1024])
            cosk = res.tile([P, N], bf16, tag="cosk")
            sink = res.tile([P, N], bf16, tag="sink")
            nc.sync.dma_start(cosk[:], cosk_d[:])
            nc.sync.dma_start(sink[:], sink_d[:])
            for k in range(KD):      # SP: col chunk 2
                nc.sync.dma_start(xkv[k][:, 1024:1536],
                                  xkv_d[k * P:(k + 1) * P, 1024:1536])
            for k in range(KD):      # Act: col chunk 3
                nc.scalar.dma_start(xkv[k][:, 1536:N],
                                    xkv_d[k * P:(k + 1) * P, 1536:N])
            load_wqk(1)

            attnT = []
            for k in range(KD):
                attnT.append(res.tile([P, NQ], bf16, tag=f"attnT{k}", name=f"attnT{k}"))
            vaug = []
            for mt in range(NKT):
                vt = res.tile([P, HPB, 2, 65], bf16, tag=f"vaug{mt}", name=f"vaug{mt}")
                nc.vector.memset(vt[:, :, :, 64], 1.0)
                vaug.append(vt)

            state = {}

            # ---- projection chunk emitters --------------------------------
            def emit_proj_chunk(wb, dst, n):
                """One 512-token projection chain into dst (qraw/kraw)."""
                ps = psA.tile([P, 512], f32, tag="ps")
                for k in range(KD):
                    nc.tensor.matmul(ps[:], wb[:, k, :],
                                     xkv[k][:, n * 512:(n + 1) * 512],
                                     start=(k == 0), stop=(k == KD - 1))
                nc.vector.tensor_copy(dst[:, n * 512:(n + 1) * 512], ps[:])

            def emit_rope_chunk(raw, rot, n):
                sl = slice(n * 512, (n + 1) * 512)
                psw = psA.tile([P, 512], f32, tag="ps")
                nc.tensor.matmul(psw[:], rm[:], raw[:, sl], start=True, stop=True)
                nc.vector.tensor_mul(rot[:, sl], raw[:, sl], cosk[:, sl])
                t2 = tmp.tile([P, 512], bf16, tag="t2")
                nc.vector.tensor_mul(t2[:], psw[:], sink[:, sl])
                nc.vector.tensor_add(rot[:, sl], rot[:, sl], t2[:])

            def emit_vproj_mt(wvt, bn, mt):
                """V-projection for kj tile mt, feat block bn."""
                ps = psA.tile([P, 512], f32, tag="ps")
                for k in range(KD):
                    nc.tensor.matmul(ps[:], xkv[k][:, mt * P:(mt + 1) * P],
                                     wvt[k][:],
                                     start=(k == 0), stop=(k == KD - 1))
                nc.vector.tensor_copy(
                    vaug[mt][:, bn * 4:(bn + 1) * 4, :, 0:64],
                    ps[:].rearrange("p (b h d) -> p b h d", b=4, h=2))

            # ---- filler generators: yield ~PE cycles emitted so far --------
            def gen_proj(hp):
                """Project+rope feat block hp (heads 2hp, 2hp+1)."""
                if hp + 1 < HPB and hp + 1 not in wload:
                    load_wqk(hp + 1)               # prefetch next block's weights
                wqb, wkb = wload.pop(hp)
                qraw = rawp.tile([P, NQ], bf16, tag="qraw")
                for n in range(NQ // 512):
                    emit_proj_chunk(wqb, qraw, n)
                    yield 4096
                qrot = rotp.tile([P, NQ], bf16, tag="qrot")
                for n in range(NQ // 512):
                    emit_rope_chunk(qraw, qrot, n)
                    yield 512
                kraw = rawp.tile([P, N], bf16, tag="kraw")
                for n in range(N // 512):
                    emit_proj_chunk(wkb, kraw, n)
                    yield 4096
                krot = rotp.tile([P, N], bf16, tag="krot")
                for n in range(N // 512):
                    emit_rope_chunk(kraw, krot, n)
                    yield 512
                state[hp] = (qrot, krot)

            wv1 = []

            def gen_vproj_hp(hp):
                """V projection for one hp block of the bn=1 feat half.

                Narrow 128-col chains so head blocks 6/7 (consumed only by
                the last AV pairs) can fill the otherwise-starved last slots.
                """
                if not wv1:
                    for k in range(KD):
                        t = wstream.tile([P, 512], bf16, tag=f"wv{k}",
                                         name=f"wv{k}")
                        nc.sync.dma_start(t[:], wv_d[k * P:(k + 1) * P, 512:1024])
                        wv1.append(t)
                    yield 0
                c0 = (hp - 4) * P
                for mt in range(NKT):
                    ps = psA.tile([P, 512], f32, tag="ps")
                    for k in range(KD):
                        nc.tensor.matmul(ps[:, 0:P],
                                         xkv[k][:, mt * P:(mt + 1) * P],
                                         wv1[k][:, c0:c0 + P],
                                         start=(k == 0), stop=(k == KD - 1))
                    nc.vector.tensor_copy(
                        vaug[mt][:, hp, :, 0:64],
                        ps[:, 0:P].rearrange("p (h d) -> p h d", h=2))
                    yield 1024

            def gen_wo_prefetch(n, tags):
                wot = []
                for k in range(KD):
                    t = wstream.tile([P, 512], bf16, tag=tags.format(k=k),
                                     name=f"wo{n}_{k}")
                    nc.sync.dma_start(t[:], wo_d[k * P:(k + 1) * P,
                                                 n * 512:(n + 1) * 512])
                    wot.append(t)
                wo_pre[n] = wot
                yield 0

            wo_pre = {}

            # ---- filler scheduler -----------------------------------------
            # (generator, deadline_slot): generator must be fully drained
            # before slot `deadline_slot` begins emitting.
            fillers = [
                (gen_proj(1), 2), (gen_proj(2), 4), (gen_proj(3), 6),
                (gen_proj(4), 8), (gen_vproj_hp(4), 9), (gen_proj(5), 10),
                (gen_vproj_hp(5), 11), (gen_proj(6), 12),
                (gen_vproj_hp(6), 13), (gen_proj(7), 14),
                (gen_wo_prefetch(0, "wo{k}"), 14),
                (gen_vproj_hp(7), 15),
                (gen_wo_prefetch(1, "wv{k}"), 16),   # reuse dead wv slots
            ]
            # total filler PE cycles: 7 proj x 27648 + vproj 65536
            filler_left = [7 * 27648 + 65536]

            credit = [0.0]

            def drive_filler(budget):
                # credit paces coarse chunks: a 4096-cycle chunk emitted on a
                # 1012-cycle budget leaves negative credit, skipping beats
                # until repaid.
                credit[0] += budget
                while fillers and credit[0] > 0:
                    gen, _ = fillers[0]
                    try:
                        c = next(gen)
                        credit[0] -= c
                        filler_left[0] -= c
                    except StopIteration:
                        fillers.pop(0)

            def drain_due(slot):
                while fillers and fillers[0][1] <= slot:
                    gen, _ = fillers[0]
                    for c in gen:
                        filler_left[0] -= c
                    fillers.pop(0)

            # ---- attention building blocks --------------------------------
            def emit_qk_beat(hp, half, kt):
                qrot, krot = state[hp]
                hoff = half * DH
                ps = psS.tile([P, NQ], f32, tag="s")
                for qn in range(NQ // 512):
                    nc.tensor.matmul(
                        ps[:, qn * 512:(qn + 1) * 512],
                        krot[hoff:hoff + DH, kt * P:(kt + 1) * P],
                        qrot[hoff:hoff + DH, qn * 512:(qn + 1) * 512],
                        start=True, stop=True)
                pt = ptp.tile([P, NQ], bf16, tag="pt")
                nc.scalar.activation(pt[:], ps[:], Exp)
                return pt

            dsb2 = {}

            def emit_av_chain(hp, half, pts, qt):
                """One qi-tile AV chain + divide; on half 1, transpose the
                paired [qi, 128] block (both halves of the head-pair) into
                attn^T with a single matmul against the identity: out[f, c]
                = sum_p dsb[p, f] * I[p, c] = dsb[c, f].  Same PE cost as
                transpose mode, but no value constraint on the rhs, and one
                transpose + one copy per TWO AV chains."""
                pv = psV.tile([P, 65], f32, tag=f"av{qt % 2}", name="av")
                for kt in range(NKT):
                    nc.tensor.matmul(pv[:], pts[kt][:, qt * P:(qt + 1) * P],
                                     vaug[kt][:, hp, half, :],
                                     start=(kt == 0), stop=(kt == NKT - 1))
                rec = small.tile([P, 1], f32, tag="rec")
                nc.vector.reciprocal(rec[:], pv[:, 64:65])
                if half == 0:
                    dsb2[qt] = dsbp.tile([P, 2, DH], bf16, tag="dsb2", name="dsb2")
                d2 = dsb2[qt]
                nc.vector.tensor_scalar_mul(d2[:, half, :], pv[:, 0:DH], rec[:])
                if half == 1:
                    pst = psA.tile([P, 512], f32, tag="ps")
                    pstv = pst[0:P, 0:P]
                    nc.tensor.matmul(pstv, d2[:].rearrange("p h d -> p (h d)"),
                                     ident[:], start=True, stop=True)
                    nc.vector.tensor_copy(
                        attnT[hp][:, qt * P:(qt + 1) * P], pstv)
                    dsb2.pop(qt)

            # ---- emission: startup, pipelined slots, tail -----------------
            # Column-gated startup: proj(0) and vproj(0) interleaved in the
            # order their xkv column chunks arrive from DRAM, so the PE
            # chases the DMA stream instead of waiting for all of xkv.
            load_wqk(2)
            wqb0, wkb0 = wload.pop(0)
            qraw = rawp.tile([P, NQ], bf16, tag="qraw")
            kraw = rawp.tile([P, N], bf16, tag="kraw")
            for n in range(N // 512):
                if n < NQ // 512:
                    emit_proj_chunk(wqb0, qraw, n)
                emit_proj_chunk(wkb0, kraw, n)
                for mt in range(4 * n, 4 * n + 4):
                    emit_vproj_mt(wv0, 0, mt)
                if n == 1:
                    qrot = rotp.tile([P, NQ], bf16, tag="qrot")
                    for nn in range(NQ // 512):
                        emit_rope_chunk(qraw, qrot, nn)
            krot = rotp.tile([P, N], bf16, tag="krot")
            for n in range(N // 512):
                emit_rope_chunk(kraw, krot, n)
            state[0] = (qrot, krot)

            prev = None                # (hp, half, pts) of pair p-1
            for p in range(2 * HPB):
                hp, half = divmod(p, 2)
                drain_due(p)
                slots_left = 2 * HPB - p
                budget = max(0, filler_left[0] // (slots_left * NKT) + 1)
                # prepump: QK kt0/kt1 reuse psS buffers still draining through
                # the previous pair's last exps; filler work is independent
                # of that, so spend some here to absorb the wait
                drive_filler(3000)
                pts = []
                for kt in range(NKT):
                    pts.append(emit_qk_beat(hp, half, kt))
                    if prev is not None and kt % 2 == 1:
                        emit_av_chain(prev[0], prev[1], prev[2], kt // 2)
                    drive_filler(budget)
                if half == 1:
                    state.pop(hp)      # krot/qrot no longer needed after QK
                prev = (hp, half, pts)
            drain_due(99)              # whatever filler is left (wo prefetch)
            for qt in range(NQT):      # last pair's AV
                emit_av_chain(prev[0], prev[1], prev[2], qt)

            # ---- out projection: out[tok, DIM] = attnT.T @ Wout -----------
            for n in range(DIM // 512):
                wot = wo_pre[n]
                for mt in range(NQ // P):
                    ps = psA.tile([P, 512], f32, tag="ps")
                    for k in range(KD):
                        nc.tensor.matmul(ps[:], attnT[k][:, mt * P:(mt + 1) * P],
                                         wot[k][:],
                                         start=(k == 0), stop=(k == KD - 1))
                    st = ostage.tile([P, 512], f32, tag="ost")
                    nc.scalar.copy(st[:], ps[:])
                    nc.sync.dma_start(
                        out_d[mt * P:(mt + 1) * P, n * 512:(n + 1) * 512], st[:])

    nc.compile()
    return nc


def _prep_inputs(x, sin, cos, Wqkv, Wout):
    """Host-side sharding/layout prep. Returns in_maps list for 8 cores."""
    x = np.asarray(x, np.float32)
    Wqkv = np.asarray(Wqkv, np.float32)
    Wout = np.asarray(Wout, np.float32)
    scale = DH ** -0.5
    wq = (Wqkv[:, :INNER] * scale).astype(BF)
    wk = Wqkv[:, INNER:2 * INNER].astype(BF)
    wv = Wqkv[:, 2 * INNER:].astype(BF)
    wo = Wout.astype(BF)
    cos_pad, sin_pad, Rm = _build_rope_consts(
        np.asarray(sin, np.float32), np.asarray(cos, np.float32))
    rm = Rm.astype(BF)
    ident = np.eye(P, dtype=np.float32).astype(BF)

    in_maps = []
    for c in range(NCORES):
        b, half = divmod(c, 2)
        xT = np.ascontiguousarray(x[b].T)                          # [DIM, N]
        ck, sk = cos_pad, sin_pad
        if half == 1:        # rotate tokens so this core's queries come first
            xT = np.concatenate([xT[:, NQ:], xT[:, :NQ]], axis=1)
            ck = np.concatenate([ck[:, NQ:], ck[:, :NQ]], axis=1)
            sk = np.concatenate([sk[:, NQ:], sk[:, :NQ]], axis=1)
        in_maps.append({
            "xkv": np.ascontiguousarray(xT).astype(BF),
            "wq": wq, "wk": wk, "wv": wv, "wo": wo,
            "cosk": np.ascontiguousarray(ck).astype(BF),
            "sink": np.ascontiguousarray(sk).astype(BF),
            "rm": rm,
            "ident": ident,
        })
    return in_maps


LAST_RESULTS = None


def kernel(x, sin, cos, Wqkv, Wout):
    global LAST_RESULTS
    if "nc" not in _CACHE:
        _CACHE["nc"] = _build_program()
    nc = _CACHE["nc"]
    in_maps = _prep_inputs(x, sin, cos, Wqkv, Wout)
    trace = bool(int(os.environ.get("KERNEL_TRACE", "0")))
    try:
        res = run_bass_kernel_spmd(nc, in_maps, core_ids=list(range(NCORES)),
                                   trace=trace)
    except (ImportError, ModuleNotFoundError):
        # NTFF profiling hook unavailable in this environment
        res = run_bass_kernel_spmd(nc, in_maps, core_ids=list(range(NCORES)),
                                   trace=False)
    LAST_RESULTS = res
    out = np.empty((B, N, DIM), np.float32)
    for c in range(NCORES):
        b, half = divmod(c, 2)
        out[b, half * NQ:(half + 1) * NQ, :] = res.results[c]["out"]
    return out
